# revision 28
# baseline (speedup 1.0000x reference)
"""Per-entity linear head: out[n, e] = sum_h x[n, e, h] * W[e, h] + b[e].

Full inputs: cell_states (4, 512, 64, 1024) f32, W (64, 1024), b (64,).
Data-parallel over the flattened batch*seq dim across 8 cores (64 MiB of
x per core); W/b are tiny and replicated, host-duplicated to 128
partitions so no on-chip broadcast is ever needed.

Per core: x_core viewed as [16384, 1024] rows.  Reduce-tile tt puts row
128*tt + p on partition p, so partition p always owns entity
e = p % 64 and W needs only a [128, 1024] resident tile.  One fused DVE
scalar_tensor_tensor per tile computes y[:, tt] = sum_h(x * w) in a
single pass over the data (the elementwise product is discarded into a
stride-0 dummy); bias is a per-partition tensor_scalar_add.

Timing model (from the perfetto trace): end-to-end is
  preamble (~7us) + x stream + last-chunk completion receipt (~2us)
  + tail compute + y store + postamble.
The x stream is SDMA-engine-bound at ~400 GB/s instantaneous (16
engines x ~27 GiB/s AXI ports; descriptor size 4KB vs 8KB makes no
difference), but a single HWDGE queue loses ~0.7us turnaround per
dma_start.  So the chunks ALTERNATE between the two HWDGE rings
(nc.sync / nc.scalar): one ring's completion turnaround overlaps the
other ring's data flow.  Structure rules measured the hard way:
- 4 MiB dma_starts are the per-ring sweet spot (G=4: 336 GB/s, G=8:
  400 GB/s, G=12: ~350 GB/s -- a ~1024-descriptor ring limit),
- at most 2 small head allocations ([2,6], one per ring) before the
  G-chunks; more gates chunk issue on STT completions and starves the
  issue pipeline,
- the last 8 tiles are single-tile dma_starts (alternating rings) into
  a dedicated zero-reuse pool: post-stream work = 1 STT, and tail
  turnarounds overlap across rings,
- w/b ride one combined [128, 1025] tensor, first on the ACT ring,
- y stores are emitted AFTER all tail dma_starts: a dma_start whose
  semaphore wait blocks mid-ring would stall every later issue on that
  ring.  Bias+store go in two pieces so the bulk store's completion
  receipt is off the critical path.

Notes:
- bacc.Bacc + nc.compile() (not raw Bass): compile() splits multi-sem
  waits into EventSemaphore instructions (walrus here allows only one
  wait per instruction) and codegens InstISA subclasses.
- The fused DVE TENSOR_TENSOR_REDUCE (InstISA) compiles but faults at
  runtime on this terminal; InstTensorScalarPtr (scalar_tensor_tensor)
  with accum_out is the native-BIR equivalent and runs fine.
- bf16 STT is SLOWER (1466ns vs 1219ns: no 2x uop for STT) -- keep
  f32.  nc.gpsimd.scalar_tensor_tensor fails walrus codegen.
- w lives in PSUM: the DVE reads it over its dedicated PSUM port,
  halving DVE's SBUF read traffic (which contends with the DMA write
  stream).  DMA can't target PSUM, so stage through SBUF and copy on
  the otherwise-idle ScalarE.
"""

import numpy as np

import concourse.bass as bass
import concourse.mybir as mybir
from concourse import bacc, bass_utils
from concourse.tile import TileContext

B, S, E, H = 4, 512, 64, 1024
N_CORES = 8
N = B * S                # 2048 flattened batch*seq rows
NPC = N // N_CORES       # 256 n-rows per core
R = NPC * E              # 16384 (n, e) rows of length H per core
P = 128                  # SBUF partitions
T = R // P               # 128 reduce tiles / output columns per core
G = 8                    # reduce tiles per main DMA (4 MiB each)
TAIL = 8                 # trailing single-tile DMAs (512 KiB each)
SPLIT = T - TAIL         # y cols [0:SPLIT] bias+store early
X_BUFS = 4


def build() -> bass.Bass:
    nc = bacc.Bacc("TRN2", target_bir_lowering=False, enable_asserts=False)
    x = nc.dram_tensor("x", [R, H], mybir.dt.float32, kind="ExternalInput")
    wb = nc.dram_tensor("wb", [P, H + 1], mybir.dt.float32, kind="ExternalInput")
    y = nc.dram_tensor("y", [P, T], mybir.dt.float32, kind="ExternalOutput")

    xt_rows = x.rearrange("(tt p) h -> tt p h", p=P)  # [T, P, H]

    with TileContext(nc) as tc:
        with (
            tc.tile_pool(name="xpool", bufs=X_BUFS) as xpool,
            tc.tile_pool(name="xtail", bufs=TAIL) as xtail,
            tc.tile_pool(name="consts", bufs=1) as consts,
            tc.tile_pool(name="wpsum", bufs=1, space="PSUM") as wpsum,
            # scratch (dummy product sink) stays in SBUF: putting it in
            # PSUM contends with the w reads on DVE's PSUM port
            tc.tile_pool(name="consts2", bufs=1) as consts2,
            tc.tile_pool(name="scratch", bufs=4) as scratch,
        ):
            wb_stage = consts.tile([P, H + 1], mybir.dt.float32)
            w_sb = wpsum.tile([P, H], mybir.dt.float32)
            y_sb = consts2.tile([P, T], mybir.dt.float32)

            # wb first on the ACT ring (prompt HWDGE completion); the
            # PSUM copy runs on ScalarE while chunk 0 is in flight
            nc.scalar.dma_start(out=wb_stage[:], in_=wb[:])
            nc.scalar.copy(w_sb[:], wb_stage[:, 0:H])

            def stt(xtile, c):
                dummy = scratch.tile([P, 1], mybir.dt.float32)
                nc.vector.scalar_tensor_tensor(
                    out=dummy.broadcast_to((P, H)),
                    in0=xtile,
                    scalar=1.0,
                    in1=w_sb[:],
                    op0=mybir.AluOpType.bypass,
                    op1=mybir.AluOpType.mult,
                    accum_out=y_sb[:, c : c + 1],
                )

            # main stream: 2+6 head (earlier first completion sem ->
            # DVE starts sooner), then G-tile chunks, all on the SP
            # ring (any splitting of the x stream across rings -- even
            # just the head -- measured 230-240us vs 192us: the rings
            # share the same 16 SDMA engines and interleaving wrecks
            # the single-queue pipelining)
            sizes = [2, 6] + [G] * ((SPLIT - 8) // G)
            start = 0
            for n in sizes:
                xt = xpool.tile([P, n, H], mybir.dt.float32, tag="xt")
                nc.sync.dma_start(
                    out=xt[:],
                    in_=xt_rows[start : start + n].rearrange("t p h -> p t h"),
                )
                for i in range(n):
                    stt(xt[:, i], start + i)
                start += n

            # bias for the bulk piece: emitted here (DVE order: right
            # after the col SPLIT-1 STT), store emitted after the tail
            # dma_starts so its semaphore wait can't stall a ring
            nc.vector.tensor_scalar_add(
                y_sb[:, 0:SPLIT], y_sb[:, 0:SPLIT], wb_stage[:, H : H + 1]
            )

            # tail: single-tile DMAs on the SP ring, zero reuse
            for c in range(SPLIT, T):
                xt1 = xtail.tile([P, H], mybir.dt.float32, tag="xt1")
                nc.sync.dma_start(out=xt1[:], in_=xt_rows[c].rearrange("p h -> p h"))
                stt(xt1[:], c)

            nc.scalar.dma_start(out=y[:, 0:SPLIT], in_=y_sb[:, 0:SPLIT])
            nc.vector.tensor_scalar_add(
                y_sb[:, SPLIT:T], y_sb[:, SPLIT:T], wb_stage[:, H : H + 1]
            )
            nc.scalar.dma_start(out=y[:, SPLIT:T], in_=y_sb[:, SPLIT:T])
    nc.compile()
    return nc


def _prepare_in_maps(cell_states, W, b):
    x_all = np.ascontiguousarray(cell_states, dtype=np.float32).reshape(N * E, H)
    w2 = np.concatenate([W, W], axis=0)                  # [128, H]
    b2 = np.concatenate([b, b]).reshape(P, 1)            # [128, 1]
    wb2 = np.ascontiguousarray(
        np.concatenate([w2, b2], axis=1), dtype=np.float32
    )
    in_maps = []
    for c in range(N_CORES):
        xc = x_all[c * R : (c + 1) * R]
        in_maps.append({"x": xc, "wb": wb2})
    return in_maps


def _unshard(per_core_y):
    outs = []
    for y_raw in per_core_y:
        # y_raw[p, tt] = out[2*tt + p//64, p%64] within the core's 256 rows
        outs.append(
            np.asarray(y_raw).reshape(2, E, T).transpose(2, 0, 1).reshape(NPC, E)
        )
    return np.concatenate(outs, axis=0).reshape(B, S, E)


def kernel_with_results(trace=False, **inputs):
    nc = build()
    in_maps = _prepare_in_maps(inputs["cell_states"], inputs["W"], inputs["b"])
    res = bass_utils.run_bass_kernel_spmd(
        nc, in_maps, core_ids=list(range(N_CORES)), trace=trace
    )
    out = _unshard([r["y"] for r in res.results])
    return out, res


def kernel(**inputs) -> np.ndarray:
    out, _ = kernel_with_results(trace=False, **inputs)
    return out


# revision 29
# speedup vs baseline: 1.0251x; 1.0251x over previous
"""Per-entity linear head: out[n, e] = sum_h x[n, e, h] * W[e, h] + b[e].

Full inputs: cell_states (4, 512, 64, 1024) f32, W (64, 1024), b (64,).
Data-parallel over the flattened batch*seq dim across 8 cores (64 MiB of
x per core); W/b are tiny and replicated, host-duplicated to 128
partitions so no on-chip broadcast is ever needed.

Per core: x_core viewed as [16384, 1024] rows.  Reduce-tile tt puts row
128*tt + p on partition p, so partition p always owns entity
e = p % 64 and W needs only a [128, 1024] resident tile.  One fused DVE
scalar_tensor_tensor per tile computes y[:, tt] = sum_h(x * w) in a
single pass over the data (the elementwise product is discarded into a
stride-0 dummy); bias is a per-partition tensor_scalar_add.

Timing model (from the perfetto trace): end-to-end is
  preamble (~7us) + x stream + last-chunk completion receipt (~2us)
  + tail compute + y store + postamble.
The x stream is SDMA-engine-bound at ~400 GB/s instantaneous (16
engines x ~27 GiB/s AXI ports; descriptor size 4KB vs 8KB makes no
difference), but a single HWDGE queue loses ~0.7us turnaround per
dma_start.  So the chunks ALTERNATE between the two HWDGE rings
(nc.sync / nc.scalar): one ring's completion turnaround overlaps the
other ring's data flow.  Structure rules measured the hard way:
- 4 MiB dma_starts are the per-ring sweet spot (G=4: 336 GB/s, G=8:
  400 GB/s, G=12: ~350 GB/s -- a ~1024-descriptor ring limit),
- at most 2 small head allocations ([2,6], one per ring) before the
  G-chunks; more gates chunk issue on STT completions and starves the
  issue pipeline,
- the last 8 tiles are single-tile dma_starts (alternating rings) into
  a dedicated zero-reuse pool: post-stream work = 1 STT, and tail
  turnarounds overlap across rings,
- w/b ride one combined [128, 1025] tensor, first on the ACT ring,
- y stores are emitted AFTER all tail dma_starts: a dma_start whose
  semaphore wait blocks mid-ring would stall every later issue on that
  ring.  Bias+store go in two pieces so the bulk store's completion
  receipt is off the critical path.

Notes:
- bacc.Bacc + nc.compile() (not raw Bass): compile() splits multi-sem
  waits into EventSemaphore instructions (walrus here allows only one
  wait per instruction) and codegens InstISA subclasses.
- The fused DVE TENSOR_TENSOR_REDUCE (InstISA) compiles but faults at
  runtime on this terminal; InstTensorScalarPtr (scalar_tensor_tensor)
  with accum_out is the native-BIR equivalent and runs fine.
- bf16 STT is SLOWER (1466ns vs 1219ns: no 2x uop for STT) -- keep
  f32.  nc.gpsimd.scalar_tensor_tensor fails walrus codegen.
- w lives in PSUM: the DVE reads it over its dedicated PSUM port,
  halving DVE's SBUF read traffic (which contends with the DMA write
  stream).  DMA can't target PSUM, so stage through SBUF and copy on
  the otherwise-idle ScalarE.
"""

import numpy as np

import concourse.bass as bass
import concourse.mybir as mybir
from concourse import bacc, bass_utils
from concourse.tile import TileContext

B, S, E, H = 4, 512, 64, 1024
N_CORES = 8
N = B * S                # 2048 flattened batch*seq rows
NPC = N // N_CORES       # 256 n-rows per core
R = NPC * E              # 16384 (n, e) rows of length H per core
P = 128                  # SBUF partitions
T = R // P               # 128 reduce tiles / output columns per core
G = 8                    # reduce tiles per main DMA (4 MiB each)
TAIL = 8                 # trailing single-tile DMAs (512 KiB each)
SPLIT = T - TAIL         # y cols [0:SPLIT] bias+store early
X_BUFS = 4


def build() -> bass.Bass:
    nc = bacc.Bacc("TRN2", target_bir_lowering=False, enable_asserts=False)
    x = nc.dram_tensor("x", [R, H], mybir.dt.float32, kind="ExternalInput")
    wb = nc.dram_tensor("wb", [P, H + 1], mybir.dt.float32, kind="ExternalInput")
    y = nc.dram_tensor("y", [P, T], mybir.dt.float32, kind="ExternalOutput")

    xt_rows = x.rearrange("(tt p) h -> tt p h", p=P)  # [T, P, H]

    with TileContext(nc) as tc:
        with (
            tc.tile_pool(name="xpool", bufs=X_BUFS) as xpool,
            tc.tile_pool(name="xtail", bufs=TAIL) as xtail,
            tc.tile_pool(name="consts", bufs=1) as consts,
            tc.tile_pool(name="wpsum", bufs=1, space="PSUM") as wpsum,
            # scratch (dummy product sink) stays in SBUF: putting it in
            # PSUM contends with the w reads on DVE's PSUM port
            tc.tile_pool(name="consts2", bufs=1) as consts2,
            tc.tile_pool(name="scratch", bufs=4) as scratch,
        ):
            wb_stage = consts.tile([P, H + 1], mybir.dt.float32)
            w_sb = wpsum.tile([P, H], mybir.dt.float32)
            y_sb = consts2.tile([P, T], mybir.dt.float32)

            # wb first on the ACT ring (prompt HWDGE completion); the
            # PSUM copy runs on ScalarE while chunk 0 is in flight
            nc.scalar.dma_start(out=wb_stage[:], in_=wb[:])
            nc.scalar.copy(w_sb[:], wb_stage[:, 0:H])

            def stt(xtile, c):
                dummy = scratch.tile([P, 1], mybir.dt.float32)
                nc.vector.scalar_tensor_tensor(
                    out=dummy.broadcast_to((P, H)),
                    in0=xtile,
                    scalar=1.0,
                    in1=w_sb[:],
                    op0=mybir.AluOpType.mult,
                    op1=mybir.AluOpType.mult,
                    accum_out=y_sb[:, c : c + 1],
                )

            # main stream: 2+6 head (earlier first completion sem ->
            # DVE starts sooner), then G-tile chunks, all on the SP
            # ring (any splitting of the x stream across rings -- even
            # just the head -- measured 230-240us vs 192us: the rings
            # share the same 16 SDMA engines and interleaving wrecks
            # the single-queue pipelining)
            sizes = [2, 6] + [G] * ((SPLIT - 8) // G)
            start = 0
            for n in sizes:
                xt = xpool.tile([P, n, H], mybir.dt.float32, tag="xt")
                nc.sync.dma_start(
                    out=xt[:],
                    in_=xt_rows[start : start + n].rearrange("t p h -> p t h"),
                )
                for i in range(n):
                    stt(xt[:, i], start + i)
                start += n

            # bias for the bulk piece: emitted here (DVE order: right
            # after the col SPLIT-1 STT), store emitted after the tail
            # dma_starts so its semaphore wait can't stall a ring
            nc.vector.tensor_scalar_add(
                y_sb[:, 0:SPLIT], y_sb[:, 0:SPLIT], wb_stage[:, H : H + 1]
            )

            # tail: single-tile DMAs on the SP ring, zero reuse
            for c in range(SPLIT, T):
                xt1 = xtail.tile([P, H], mybir.dt.float32, tag="xt1")
                nc.sync.dma_start(out=xt1[:], in_=xt_rows[c].rearrange("p h -> p h"))
                stt(xt1[:], c)

            nc.scalar.dma_start(out=y[:, 0:SPLIT], in_=y_sb[:, 0:SPLIT])
            nc.vector.tensor_scalar_add(
                y_sb[:, SPLIT:T], y_sb[:, SPLIT:T], wb_stage[:, H : H + 1]
            )
            nc.scalar.dma_start(out=y[:, SPLIT:T], in_=y_sb[:, SPLIT:T])
    nc.compile()
    return nc


def _prepare_in_maps(cell_states, W, b):
    x_all = np.ascontiguousarray(cell_states, dtype=np.float32).reshape(N * E, H)
    w2 = np.concatenate([W, W], axis=0)                  # [128, H]
    b2 = np.concatenate([b, b]).reshape(P, 1)            # [128, 1]
    wb2 = np.ascontiguousarray(
        np.concatenate([w2, b2], axis=1), dtype=np.float32
    )
    in_maps = []
    for c in range(N_CORES):
        xc = x_all[c * R : (c + 1) * R]
        in_maps.append({"x": xc, "wb": wb2})
    return in_maps


def _unshard(per_core_y):
    outs = []
    for y_raw in per_core_y:
        # y_raw[p, tt] = out[2*tt + p//64, p%64] within the core's 256 rows
        outs.append(
            np.asarray(y_raw).reshape(2, E, T).transpose(2, 0, 1).reshape(NPC, E)
        )
    return np.concatenate(outs, axis=0).reshape(B, S, E)


def kernel_with_results(trace=False, **inputs):
    nc = build()
    in_maps = _prepare_in_maps(inputs["cell_states"], inputs["W"], inputs["b"])
    res = bass_utils.run_bass_kernel_spmd(
        nc, in_maps, core_ids=list(range(N_CORES)), trace=trace
    )
    out = _unshard([r["y"] for r in res.results])
    return out, res


def kernel(**inputs) -> np.ndarray:
    out, _ = kernel_with_results(trace=False, **inputs)
    return out
